# revision 8
# baseline (speedup 1.0000x reference)
"""Trainium2 Bass kernel for nn_DiGCNLayerAtt (directed GCN layer with
adjacency-masked attention), batch-parallel over 8 NeuronCores.

Math (same scaling tricks as before, validated vs reference):
  softmax denominator / renormalization / row-max are uniform positive
  per-row scalings of att; LayerNorm is invariant to them. So the kernel
  computes attu[n,m] = exp(u_raw[n,m]/16) * A[n,m] and LayerNorms the
  unnormalized context  ctx = attu_left @ hl + diag(attu)*hs + attu_right @ hr.

v2 structure — exploits u's symmetry (u = h h^T):
  * u is computed only for upper-triangular 128-block cells (m-block <=
    n-block): 40 of 64 (m, chunk) cells. exp runs only on those cells
    (ACT work -37%).
  * The lower-triangle stationaries attu^T[m,n] = e[m,n]*A[n,m] (m>n) are
    rebuilt from the upper cells' e via batched DMA-xbar 128x128 block
    transposes (dma_start_transpose, 16 blocks per dispatch, 6 dispatches
    total) — no PE/DVE cost — then masked against A^T on DVE.
  * u's diagonal (|h_n|^2) is precomputed on the host (exact), so no
    on-device diag extraction; the diag block of e overflows to inf in
    f16, and strict-triangle copy_predicated masks (never touching the
    diagonal) keep inf/NaN out of every matmul operand.
  * mask-multiplies are batched 4 rows at a time ([128,2048] DVE ops).
  * SBUF-only elementwise work (self-term, variance pass, normalize pass)
    is offloaded to the Pool engine (GPSIMD cannot touch PSUM).
  * ctx staging + output are f16 (adds ~5e-4 rel err, fine vs 2e-2 tol).

Per-core layout (batch b on core b), chunk = 512 n-cols, group g = 4
m-blocks:  hdir for all m; then per group g: u-cells (c-major), et
transposes, ctx chunk g (direct gg<=g from e_all, lower gg>g from et),
evacuation+stats; finally LN + relu + f16 store.
"""
import sys

sys.path.insert(0, "/opt/trn_rl_repo")

import numpy as np

import concourse.bass as bass
import concourse.tile as tile
from concourse import bacc, mybir
from concourse import bass_utils
from concourse.bass_interp import get_hw_module

F32 = mybir.dt.float32
F16 = mybir.dt.float16
ALU = mybir.AluOpType
ACTF = mybir.ActivationFunctionType
AX = mybir.AxisListType

B, N, H = 8, 2048, 256
NT = N // 128           # 16 m/n blocks of 128
CHUNK = 512
NG = 4                  # groups of 4 blocks; also number of chunks
GS = NT // NG           # 4 blocks per group
TEMPER_INV = 1.0 / float(np.sqrt(H))  # 1/16
EPS_LN = 1e-12

# e_all column layout: group g (cells with chunk index c'=g) holds cells
# (alpha, g) for alpha = 0..4g+3, each 512 wide, at base[g] + alpha*512.
E_BASE = [0, 2048, 6144, 12288]
E_COLS = 20480

# mask-multiplies routed to the (slow but otherwise idle) GPSIMD engine:
# picked to be late-consumed within their chunk so Pool latency hides.
POOL_MASKS = {(0, 3), (0, 2), (1, 3), (1, 2), (2, 3)}


def build_program(apply_ln: bool, repeat: int = 1):
    nc = bacc.Bacc("TRN2", target_bir_lowering=False, debug=False, num_devices=B)

    hT_d = nc.dram_tensor("hT", [H, N], F16, kind="ExternalInput")
    AT_d = nc.dram_tensor("AT", [N, N], F16, kind="ExternalInput")
    wcat_d = nc.dram_tensor("wcat", [H, 3 * H], F16, kind="ExternalInput")
    bias_d = nc.dram_tensor("bias_cat", [128, 3 * H], F32, kind="ExternalInput")
    masklo_d = nc.dram_tensor("masklo", [128, 128], mybir.dt.uint8, kind="ExternalInput")
    maskup_d = nc.dram_tensor("maskup", [128, 128], mybir.dt.uint8, kind="ExternalInput")
    adiag_d = nc.dram_tensor("adiag", [128, NT], F32, kind="ExternalInput")
    udiag_d = nc.dram_tensor("udiag", [128, NT], F32, kind="ExternalInput")
    if apply_ln:
        lnw_d = nc.dram_tensor("lnw_bc", [128, H], F32, kind="ExternalInput")
        lnb_d = nc.dram_tensor("lnb_bc", [128, H], F32, kind="ExternalInput")
    out_d = nc.dram_tensor("out", [N, H], F16, kind="ExternalOutput")

    with tile.TileContext(nc) as tc:
        with (
            tc.tile_pool(name="consts", bufs=1) as cpool,
            tc.tile_pool(name="attup", bufs=4) as apool,
            tc.tile_pool(name="etp", bufs=2) as etpool,
            tc.tile_pool(name="small", bufs=4) as smpool,
            tc.tile_pool(name="outp", bufs=3) as opool,
            tc.tile_pool(name="upsum", bufs=3, space=bass.MemorySpace.PSUM) as upool,
            tc.tile_pool(name="ctxpsum", bufs=2, space=bass.MemorySpace.PSUM) as xpool,
        ):
            v = nc.vector
            sc = nc.scalar
            g_ = nc.gpsimd

            # ---- constants / persistent SBUF ----
            hT0 = cpool.tile([128, N], F16, tag="hT0")
            hT1 = cpool.tile([128, N], F16, tag="hT1")
            nc.sync.dma_start(hT0[:], hT_d.ap()[0:128, :])
            nc.sync.dma_start(hT1[:], hT_d.ap()[128:256, :])
            wcat0 = cpool.tile([128, 3 * H], F16, tag="wcat0")
            wcat1 = cpool.tile([128, 3 * H], F16, tag="wcat1")
            nc.sync.dma_start(wcat0[:], wcat_d.ap()[0:128, :])
            nc.sync.dma_start(wcat1[:], wcat_d.ap()[128:256, :])
            bias_cat = cpool.tile([128, 3 * H], F32, tag="bias_cat")
            nc.sync.dma_start(bias_cat[:], bias_d.ap())
            masklo = cpool.tile([128, 128], mybir.dt.uint8, tag="masklo")
            maskup = cpool.tile([128, 128], mybir.dt.uint8, tag="maskup")
            nc.sync.dma_start(masklo[:], masklo_d.ap())
            nc.sync.dma_start(maskup[:], maskup_d.ap())
            adiag = cpool.tile([128, NT], F32, tag="adiag")
            udiag = cpool.tile([128, NT], F32, tag="udiag")
            nc.sync.dma_start(adiag[:], adiag_d.ap())
            nc.sync.dma_start(udiag[:], udiag_d.ap())
            if apply_ln:
                lnw = cpool.tile([128, H], F32, tag="lnw")
                lnb = cpool.tile([128, H], F32, tag="lnb")
                nc.sync.dma_start(lnw[:], lnw_d.ap())
                nc.sync.dma_start(lnb[:], lnb_d.ap())

            at_all = cpool.tile([128, NT * N], F16, tag="at_all")
            hlsr = [cpool.tile([128, 3 * H], F16, tag=f"hlsr{m}", name=f"hlsr{m}")
                    for m in range(NT)]
            hl = [t[:, 0:H] for t in hlsr]
            hs = [t[:, H:2 * H] for t in hlsr]
            hr = [t[:, 2 * H:3 * H] for t in hlsr]
            e_all = cpool.tile([128, E_COLS], F16, tag="e_all")
            ctx_sb = cpool.tile([128, NT * H], F32, tag="ctx_sb")
            sum_b = cpool.tile([128, NT], F32, tag="sum_b")
            mu_b = cpool.tile([128, NT], F32, tag="mu_b")
            var_b = cpool.tile([128, NT], F32, tag="var_b")

            # AT panel views: [p, beta, n] and [p, beta, nblock, q]
            at3 = at_all[:].rearrange("p (b n) -> p b n", n=N)
            at4 = at_all[:].rearrange("p (b a q) -> p b a q", a=NT, q=128)

            def emit_all():
                # ---- AT panel loads (SP ring) ----
                for m in range(NT):
                    nc.sync.dma_start(at_all[:, m * N:(m + 1) * N],
                                      AT_d.ap()[m * 128:(m + 1) * 128, :])

                # ---- phase A: hdir for all m ----
                for m in range(NT):
                    ms = bass.ts(m, 128)
                    hp = xpool.tile([128, 2 * CHUNK], F32, tag="ctx",
                                    name=f"hp{m}")
                    nc.tensor.matmul(hp[:, 0:512], hT0[:, ms], wcat0[:, 0:512],
                                     start=True, stop=False)
                    nc.tensor.matmul(hp[:, 512:768], hT0[:, ms],
                                     wcat0[:, 512:768], start=True, stop=False)
                    nc.tensor.matmul(hp[:, 0:512], hT1[:, ms], wcat1[:, 0:512],
                                     start=False, stop=True)
                    nc.tensor.matmul(hp[:, 512:768], hT1[:, ms],
                                     wcat1[:, 512:768], start=False, stop=True)
                    v.tensor_tensor(hlsr[m][:], hp[:, 0:768], bias_cat[:],
                                    op=ALU.add)

                # de/dv for self term (tiny, do once)
                de = smpool.tile([128, NT], F32, tag="de")
                sc.activation(de[:], udiag[:], ACTF.Exp, scale=TEMPER_INV)
                dv = smpool.tile([128, NT], F32, tag="dv")
                v.tensor_tensor(dv[:], de[:], adiag[:], op=ALU.mult)

                et_tiles = {}
                for g in range(NG):
                    # ---- u-cells for group g (c-major) + et transposes ----
                    for c in range(g, NG):
                        for al in range(GS):
                            a = g * GS + al
                            as_ = bass.ts(a, 128)
                            u = upool.tile([128, CHUNK], F32, tag="u")
                            nc.tensor.matmul(u[:], hT0[:, as_],
                                             hT0[:, c * CHUNK:(c + 1) * CHUNK],
                                             start=True, stop=False)
                            nc.tensor.matmul(u[:], hT1[:, as_],
                                             hT1[:, c * CHUNK:(c + 1) * CHUNK],
                                             start=False, stop=True)
                            off = E_BASE[c] + a * CHUNK
                            sc.activation(e_all[:, off:off + CHUNK], u[:],
                                          ACTF.Exp, scale=TEMPER_INV)
                        if c > g:
                            # blocks E_{alpha in g, beta in group c} transposed
                            et = etpool.tile([128, 4 * CHUNK], F16, tag="et",
                                             name=f"et{g}_{c}")
                            sl = E_BASE[c] + g * GS * CHUNK
                            nc.sync.dma_start_transpose(
                                et[:].rearrange("p (j q) -> p j q", q=128),
                                e_all[:, sl:sl + 4 * CHUNK])
                            et_tiles[g, c] = et

                    # ---- ctx chunk g ----
                    ctx = xpool.tile([128, 4 * H], F32, tag="ctx",
                                     name=f"ctx{g}")
                    attus = []
                    for gg in range(NG):
                        attu = apool.tile([128, 4 * CHUNK], F16, tag="attu",
                                          name=f"attu{g}_{gg}")
                        if gg <= g:
                            # direct: cells (beta in gg, chunk g) from e_all
                            sl = E_BASE[g] + gg * GS * CHUNK
                            ev = e_all[:, sl:sl + 4 * CHUNK].rearrange(
                                "p (b c) -> p b c", c=CHUNK)
                            av = at3[:, gg * GS:(gg + 1) * GS,
                                     g * CHUNK:(g + 1) * CHUNK]
                            ov = attu[:].rearrange("p (b c) -> p b c", c=CHUNK)
                            meng = g_ if (g, gg) in POOL_MASKS else v
                            meng.tensor_tensor(ov, ev, av, op=ALU.mult)
                        else:
                            # lower: transposed blocks * A^T
                            et = et_tiles[g, gg]
                            ev = et[:].rearrange("p (a b q) -> p a b q",
                                                 b=GS, q=128)
                            av = at4[:, gg * GS:(gg + 1) * GS,
                                     g * GS:(g + 1) * GS, :].rearrange(
                                "p b a q -> p a b q")
                            ov = attu[:].rearrange("p (a b q) -> p a b q",
                                                   b=GS, q=128)
                            meng = g_ if (g, gg) in POOL_MASKS else v
                            meng.tensor_tensor(ov, ev, av, op=ALU.mult)
                        attus.append(attu)

                    for beta in range(NT):
                        gg, bl = beta // GS, beta % GS
                        attu = attus[gg]
                        if gg <= g:
                            def st(ns, attu=attu, bl=bl):
                                o = bl * CHUNK + ns * 128
                                return attu[:, o:o + 128]
                        else:
                            def st(ns, attu=attu, bl=bl):
                                o = (ns * GS + bl) * 128
                                return attu[:, o:o + 128]
                        # beta's own diagonal block is n-block == beta,
                        # i.e. ns == beta - g*GS, only when gg == g.
                        dns = beta - g * GS if gg == g else -1
                        for ns in range(GS):
                            nt_i = g * GS + ns
                            first = (beta == 0) and (ns % 2 == 0)
                            last = (beta == NT - 1) and (ns % 2 == 1)
                            o = ctx[:, ns * H:(ns + 1) * H]
                            if ns == dns:
                                azt = smpool.tile([128, 256], F16, tag="az",
                                                  name=f"az{g}_{beta}")
                                g_.memset(azt[:], 0)
                                alo = azt[:, 0:128]
                                aup = azt[:, 128:256]
                                v.copy_predicated(alo, masklo[:], st(ns))
                                v.copy_predicated(aup, maskup[:], st(ns))
                                nc.tensor.matmul(o, alo, hl[beta][:],
                                                 start=first, stop=False)
                                nc.tensor.matmul(o, aup, hr[beta][:],
                                                 start=False, stop=last)
                            elif beta > nt_i:
                                nc.tensor.matmul(o, st(ns), hl[beta][:],
                                                 start=first, stop=last)
                            else:
                                nc.tensor.matmul(o, st(ns), hr[beta][:],
                                                 start=first, stop=last)

                    # ---- phase C: evacuate + self term + stats ----
                    for ns in range(GS):
                        nt_i = g * GS + ns
                        tmpd = smpool.tile([128, H], F32, tag="tmpd")
                        v.tensor_scalar(tmpd[:], hs[nt_i][:],
                                        dv[:, nt_i:nt_i + 1], None,
                                        op0=ALU.mult)
                        cs = ctx_sb[:, nt_i * H:(nt_i + 1) * H]
                        v.scalar_tensor_tensor(cs, ctx[:, ns * H:(ns + 1) * H],
                                               1.0, tmpd[:], op0=ALU.mult,
                                               op1=ALU.add,
                                               accum_out=sum_b[:, nt_i:nt_i + 1])
                        v.tensor_scalar(mu_b[:, nt_i:nt_i + 1],
                                        sum_b[:, nt_i:nt_i + 1], 1.0 / H, None,
                                        op0=ALU.mult)
                        sq = smpool.tile([128, H], F32, tag="sq")
                        v.scalar_tensor_tensor(sq[:], cs,
                                               mu_b[:, nt_i:nt_i + 1], cs,
                                               op0=ALU.subtract, op1=ALU.mult,
                                               accum_out=var_b[:, nt_i:nt_i + 1])

                # ---- phase D: LayerNorm + relu + store ----
                veps = smpool.tile([128, NT], F32, tag="veps")
                v.tensor_scalar(veps[:], var_b[:], 1.0 / H, EPS_LN,
                                op0=ALU.mult, op1=ALU.add)
                sig = smpool.tile([128, NT], F32, tag="sig")
                sc.activation(sig[:], veps[:], ACTF.Sqrt)
                inv0 = smpool.tile([128, NT], F32, tag="inv0")
                v.reciprocal(inv0[:], sig[:])
                nw1 = smpool.tile([128, NT], F32, tag="nw1")
                v.tensor_tensor(nw1[:], inv0[:], inv0[:], op=ALU.mult)
                v.tensor_tensor(nw1[:], veps[:], nw1[:], op=ALU.mult)
                v.tensor_scalar(nw1[:], nw1[:], -0.5, 1.5, op0=ALU.mult,
                                op1=ALU.add)
                istd = smpool.tile([128, NT], F32, tag="istd")
                v.tensor_tensor(istd[:], inv0[:], nw1[:], op=ALU.mult)
                nmu = smpool.tile([128, NT], F32, tag="nmu")
                v.scalar_tensor_tensor(nmu[:], mu_b[:], -1.0, istd[:],
                                       op0=ALU.mult, op1=ALU.mult)
                out_sb = cpool.tile([128, NT * H], F16, tag="out_sb")
                out_v = out_d.ap().rearrange("(t p) h -> p t h", p=128)
                out_sv = out_sb[:].rearrange("p (t h) -> p t h", h=H)
                for nt_i in range(NT):
                    if apply_ln:
                        pre = opool.tile([128, H], F32, tag="pre")
                        v.tensor_scalar(pre[:], ctx_sb[:, nt_i * H:(nt_i + 1) * H],
                                        mu_b[:, nt_i:nt_i + 1],
                                        istd[:, nt_i:nt_i + 1],
                                        op0=ALU.subtract, op1=ALU.mult)
                        v.tensor_tensor(pre[:], pre[:], lnw[:], op=ALU.mult)
                        v.tensor_tensor(pre[:], pre[:], lnb[:], op=ALU.add)
                        sc.activation(out_sb[:, nt_i * H:(nt_i + 1) * H], pre[:],
                                      ACTF.Relu)
                    else:
                        sc.activation(out_sb[:, nt_i * H:(nt_i + 1) * H],
                                      ctx_sb[:, nt_i * H:(nt_i + 1) * H],
                                      ACTF.Relu, bias=nmu[:, nt_i:nt_i + 1],
                                      scale=istd[:, nt_i:nt_i + 1])
                    if nt_i % 4 == 3:
                        gi = nt_i // 4
                        nc.sync.dma_start(out_v[:, gi * 4:(gi + 1) * 4, :],
                                          out_sv[:, gi * 4:(gi + 1) * 4, :])

            for _rep in range(repeat):
                emit_all()

    nc.compile()
    nc.m = get_hw_module(nc.m)
    return nc


_cache = {}


def _get_program(apply_ln: bool):
    if apply_ln not in _cache:
        _cache[apply_ln] = build_program(apply_ln)
    return _cache[apply_ln]


def _prep_in_maps(hidden_state, adjacency_matrix, Wl, bl, Ws, bs, Wr, br,
                  ln_w, ln_b, apply_ln):
    f16 = np.float16
    tri = np.tri(128, 128, -1)
    wcat = np.concatenate([np.ascontiguousarray(Wl.T),
                           np.ascontiguousarray(Ws.T),
                           np.ascontiguousarray(Wr.T)], axis=1).astype(f16)
    bias_cat = np.concatenate([np.asarray(bl), np.asarray(bs),
                               np.asarray(br)]).astype(np.float32)
    shared = {
        "wcat": wcat,
        "bias_cat": np.broadcast_to(bias_cat, (128, 3 * H)).copy(),
        "masklo": tri.astype(np.uint8),   # [p,q]: 1 if p>q (m>n: left)
        "maskup": tri.T.astype(np.uint8), # 1 if p<q (m<n: right)
    }
    if apply_ln:
        shared["lnw_bc"] = np.broadcast_to(ln_w.astype(np.float32), (128, H)).copy()
        shared["lnb_bc"] = np.broadcast_to(ln_b.astype(np.float32), (128, H)).copy()
    A16 = np.asarray(adjacency_matrix).astype(f16)
    h32 = np.asarray(hidden_state, dtype=np.float32)
    h16 = h32.astype(f16)
    in_maps = []
    for b in range(B):
        diag = np.diagonal(adjacency_matrix[b]).astype(np.float32)
        ud = (h16[b].astype(np.float32) ** 2).sum(axis=1)
        m = dict(shared)
        m["hT"] = np.ascontiguousarray(h16[b].T)
        m["AT"] = np.ascontiguousarray(A16[b].T)
        m["adiag"] = np.ascontiguousarray(diag.reshape(NT, 128).T)
        m["udiag"] = np.ascontiguousarray(ud.reshape(NT, 128).T.astype(np.float32))
        in_maps.append(m)
    return in_maps


def kernel(hidden_state, adjacency_matrix, Wl, bl, Ws, bs, Wr, br, ln_w, ln_b):
    apply_ln = not (np.all(np.asarray(ln_w) == 1.0)
                    and np.all(np.asarray(ln_b) == 0.0))
    nc = _get_program(apply_ln)
    in_maps = _prep_in_maps(hidden_state, adjacency_matrix, Wl, bl, Ws, bs,
                            Wr, br, ln_w, ln_b, apply_ln)
    res = bass_utils.run_bass_kernel_spmd(nc, in_maps, core_ids=list(range(B)))
    return np.stack([res.results[b]["out"] for b in range(B)]).astype(np.float32)
